# revision 71
# baseline (speedup 1.0000x reference)
"""Trainium2 Bass kernel for nn_PluggableMHA (hybrid dense-synthesizer MHA).

Sharding: 8 cores = (batch b in {0,1}) x (head-group g in {0..3}, 4 heads each).
Each core computes, for its batch and its 4 heads:
    Q/K projections in transposed layout  QT/KT [d, t]
    V projection in natural layout        V  [k, d]   (+ all-ones block)
    hT = gelu(Ws1^T x_q^T + bs1)          [sh, q]
    scoresT[k, q] = s_h*(K Q^T) + (h^T Ws2)^T    accumulated in PSUM
    expT = Exp(g_h * scoresT + g_h*bs2 + pad)    (causal handled by trimming
                                                  + triangular mask tile)
    ctx'^T, sums = V_aug^T expT   (ones column block gives softmax sums)
    ctxT = ctx'^T / sums + bv
    partial_out = ctxT^T Wo_rows
The host sums the 4 head-group partials per batch and adds bo.

All matmuls run in bf16 (fp32 PSUM accumulation).
"""

import sys

if "/opt/trn_rl_repo" not in sys.path:
    sys.path.insert(0, "/opt/trn_rl_repo")

import numpy as np
import ml_dtypes

BF16 = ml_dtypes.bfloat16

# Problem shapes (hardcoded per contract).
B = 2
TQ = 1024
TK = 1024
DM = 1024          # d_model
NH = 16            # total heads
DH = 64            # head dim
SH = 1024          # synth hidden
HPC = 4            # heads per core
DCORE = HPC * DH   # 256 q/k/v columns per core
KC = HPC * TK      # 4096 ws2 columns per core
N_CORES = 8
NEG = -1e30
ATT_SCALE = DH ** -0.5

# Shard the synthesizer-hidden projection across the 4 cores of a batch and
# AllGather the result.  Saves ~25us of PE work per core, but in-NEFF
# collectives proved unreliable on the axon PJRT path (device unrecoverable),
# so this stays off.
SHARD_HT = False
SH_LOC = SH // 4 if SHARD_HT else SH

_STATE: dict = {}


def _patch_tile_drain():
    """This neuronxcc's Drain lowering supports fewer sync-wait slots than
    Tile's exit drain carries; split the waits across single-wait SP nops."""
    import bass_rust
    import concourse.tile as tile_mod
    from concourse.vector_clock import ScopedClock

    if getattr(tile_mod.TileContext, "_drain_patched", False):
        return

    def _drain_and_barrier(self, tick_clock, wait_clock):
        nc = self.nc
        drain_inst = nc.sync.drain()
        wait_clock.add_sem_waits(
            drain_inst.ins, ScopedClock({None: tick_clock.global_clock})
        )
        si = drain_inst.ins.sync_info
        waits = list(si.on_wait) if si is not None else []
        ups = list(si.on_update) if si is not None else []
        if len(waits) > 1:
            drain_inst.ins.sync_info = bass_rust.SyncInfo(on_wait=[], on_update=ups)
            for w in waits:
                n = nc.sync.nop(nofuse=True)
                n.ins.sync_info = bass_rust.SyncInfo(on_wait=[w], on_update=[])
        nc.all_engine_barrier()
        popped = nc._tile_sem_poison_stack.pop()
        assert popped is self._sem_poison
        nc.clear_and_free_semaphores(list(self.sems.allocated().values()))
        nc.all_engine_barrier()

    tile_mod.TileContext._drain_and_barrier = _drain_and_barrier
    tile_mod.TileContext._drain_patched = True


def _split_excess_waits(nc, max_waits=1):
    """This neuronxcc build supports only a small number of sync-wait slots
    per instruction; hoist excess waits onto prepended same-engine nops."""
    import concourse.mybir as mybir
    import bass_rust

    cnt = 0

    def fix_block(b):
        nonlocal cnt
        new_insts = []
        changed = False
        for inst in b.instructions:
            si = inst.sync_info
            waits = list(si.on_wait) if si is not None else []
            if len(waits) > max_waits:
                extra, keep = waits[:-max_waits], waits[-max_waits:]
                for w in extra:
                    n = mybir.InstNoOp(name=f"{inst.name}-wsplit{cnt}", ins=[], outs=[])
                    cnt += 1
                    n.engine = inst.engine
                    n.sync_info = bass_rust.SyncInfo(on_wait=[w], on_update=[])
                    new_insts.append(n)
                inst.sync_info = bass_rust.SyncInfo(
                    on_wait=keep, on_update=list(si.on_update))
                changed = True
            new_insts.append(inst)
        if changed:
            b.instructions = new_insts
        for sb in getattr(b, "blocks", None) or []:
            fix_block(sb)

    for f in nc.m.functions:
        for b in f.blocks:
            fix_block(b)
    return cnt


def _build(reps=1, loop_n=1):
    import concourse.bass as bass
    import concourse.tile as tile
    import concourse.mybir as mybir
    from contextlib import ExitStack

    _patch_tile_drain()

    dt = mybir.dt
    AF = mybir.ActivationFunctionType
    ALU = mybir.AluOpType

    nc = bass.Bass()

    # ---- DRAM I/O -------------------------------------------------------
    xqT_d = nc.declare_dram_parameter("xqT", [DM, TQ], dt.bfloat16, isOutput=False)
    xkvT_d = nc.declare_dram_parameter("xkvT", [DM, TK], dt.bfloat16, isOutput=False)
    wq_d = nc.declare_dram_parameter("wq", [DM, DCORE], dt.bfloat16, isOutput=False)
    wk_d = nc.declare_dram_parameter("wk", [DM, DCORE], dt.bfloat16, isOutput=False)
    wv_d = nc.declare_dram_parameter("wv", [DM, DCORE], dt.bfloat16, isOutput=False)
    ws1_d = nc.declare_dram_parameter("ws1", [DM, SH_LOC], dt.bfloat16, isOutput=False)
    ws2_d = nc.declare_dram_parameter("ws2", [SH, KC], dt.bfloat16, isOutput=False)
    wo_d = nc.declare_dram_parameter("wo", [DCORE, DM], dt.bfloat16, isOutput=False)
    # small tensors packed host-side into one [128, 54] f32 block:
    # cols 0:2 bq, 2:4 bk, 4:6 bv, 6:14 bs1, 14:46 bs2 (h-major), 46:54 kpm(f32)
    small_d = nc.declare_dram_parameter("small", [128, 54], dt.float32, isOutput=False)
    gates_d = nc.declare_dram_parameter("gates", [1, HPC], dt.float32, isOutput=False)
    out_d = nc.declare_dram_parameter("out", [TQ, DM], dt.float32, isOutput=True)

    with tile.TileContext(nc) as tc, ExitStack() as ctx:
        const = ctx.enter_context(tc.tile_pool(name="const", bufs=1))

        def ctile(shape, dtype, tag):
            return const.tile(shape, dtype, tag=tag, name=tag)

        # ---- resident SBUF tensors --------------------------------------
        # DMA'd in per-Dt slices, emitted in consumption order, so the
        # projection matmuls can start as soon as their first slices land.
        gate_in = ctile([128, HPC], dt.float32, "gate_in")
        nc.sync.dma_start(out=gate_in[0:1, :], in_=gates_d[:])

        xqT = ctile([128, 8, TQ], dt.bfloat16, "xqT")
        xkvT = ctile([128, 8, TK], dt.bfloat16, "xkvT")
        wq = ctile([128, 8, DCORE], dt.bfloat16, "wq")
        wk = ctile([128, 8, DCORE], dt.bfloat16, "wk")
        wv = ctile([128, 8, DCORE], dt.bfloat16, "wv")
        ws1 = ctile([128, 8, SH_LOC], dt.bfloat16, "ws1")
        xqT_r = xqT_d.rearrange("(t p) q -> p t q", p=128)
        xkvT_r = xkvT_d.rearrange("(t p) q -> p t q", p=128)
        wq_r = wq_d.rearrange("(t p) c -> p t c", p=128)
        wk_r = wk_d.rearrange("(t p) c -> p t c", p=128)
        wv_r = wv_d.rearrange("(t p) c -> p t c", p=128)
        ws1_r = ws1_d.rearrange("(t p) s -> p t s", p=128)
        for Dt in range(8):
            nc.sync.dma_start(out=ws1[:, Dt, :], in_=ws1_r[:, Dt, :])
            nc.sync.dma_start(out=xqT[:, Dt, :], in_=xqT_r[:, Dt, :])
            if Dt == 7:
                small = ctile([128, 54], dt.float32, "small")
                nc.sync.dma_start(out=small[:], in_=small_d[:])
        bq = small[:, 0:2]
        bk = small[:, 2:4]
        bv = small[:, 4:6]
        bs1 = small[:, 6:14]
        bs2 = small[:, 14:46].rearrange("p (h j) -> p h j", h=HPC)
        kpmf = small[:, 46:54]
        nc.sync.dma_start(out=wq[:], in_=wq_r[:])
        for half in range(2):
            nc.sync.dma_start(out=wk[:, half * 4:(half + 1) * 4, :],
                              in_=wk_r[:, half * 4:(half + 1) * 4, :])
            nc.sync.dma_start(out=xkvT[:, half * 4:(half + 1) * 4, :],
                              in_=xkvT_r[:, half * 4:(half + 1) * 4, :])
        nc.sync.dma_start(out=wv[:], in_=wv_r[:])
        wo = ctile([128, 2, DM], dt.bfloat16, "wo")
        nc.sync.dma_start(out=wo[:], in_=wo_d.rearrange("(t p) n -> p t n", p=128))

        # ---- setup: per-head scalars, masks -----------------------------
        ones_t = ctile([128, 128], dt.float32, "ones_t")
        nc.vector.memset(ones_t[:], 1.0)

        with tc.tile_pool(name="gpp", bufs=1, space="PSUM") as gpp:
            ps_g = gpp.tile([128, 512], dt.float32, tag="gproj", name="ps_g")
            nc.tensor.matmul(ps_g[:, 0:HPC], lhsT=ones_t[0:1, :],
                             rhs=gate_in[0:1, :], start=True, stop=True)
            g_all = ctile([128, HPC], dt.float32, "g_all")   # sigmoid(gate), bcast
            nc.scalar.activation(g_all[:], ps_g[:, 0:HPC], AF.Sigmoid)

        r_all = ctile([128, HPC], dt.float32, "r_all")       # 1/g
        nc.vector.reciprocal(r_all[:], g_all[:])
        s_all = ctile([128, HPC], dt.float32, "s_all")       # 0.125*(1-g)/g
        nc.vector.tensor_scalar(s_all[:], r_all[:], ATT_SCALE, -ATT_SCALE,
                                op0=ALU.mult, op1=ALU.add)

        qscale = ctile([128, 2], dt.float32, "qscale")       # per-partition Q scale
        for t in range(2):
            nc.vector.tensor_copy(qscale[0:64, t:t + 1], s_all[0:64, 2 * t:2 * t + 1])
            nc.vector.tensor_copy(qscale[64:128, t:t + 1],
                                  s_all[64:128, 2 * t + 1:2 * t + 2])
        qbias = ctile([128, 2], dt.float32, "qbias")         # bq * qscale
        nc.vector.tensor_mul(qbias[:], bq[:], qscale[:])

        pad_add = ctile([128, 8], dt.float32, "pad_add")     # 0 keep / NEG masked
        nc.vector.tensor_scalar(pad_add[:], kpmf[:], -NEG, NEG,
                                op0=ALU.mult, op1=ALU.add)
        ebias = ctile([128, 4, 8], dt.float32, "ebias")      # g*bs2 + pad
        for h in range(HPC):
            nc.vector.scalar_tensor_tensor(
                out=ebias[:, h, :], in0=bs2[:, h, :], scalar=g_all[:, h:h + 1],
                in1=pad_add[:], op0=ALU.mult, op1=ALU.add)

        dmask = ctile([128, 128], dt.float32, "dmask")       # scoresT tri mask
        nc.gpsimd.memset(dmask[:], 0.0)
        # keep (0) where q - k >= 0, else NEG   (p = k, f = q within tile)
        nc.gpsimd.affine_select(out=dmask[:], in_=dmask[:],
                                compare_op=ALU.is_ge, fill=NEG, base=0,
                                pattern=[[1, 128]], channel_multiplier=-1)

        # ---- persistent compute tensors ---------------------------------
        QT = ctile([128, 2, TQ], dt.bfloat16, "QT")
        KT = ctile([128, 2, TK], dt.bfloat16, "KT")
        hT = ctile([128, 8, TQ], dt.bfloat16, "hT")
        hT_sh = ctile([128, 2, TQ], dt.bfloat16, "hT_sh")
        vaug = ctile([128, 32, 128], dt.bfloat16, "vaug")
        ctxT = ctile([128, 2, TQ], dt.bfloat16, "ctxT")

        shard_d = full_d = None
        if SHARD_HT:
            dramp = ctx.enter_context(tc.tile_pool(name="dramp", bufs=1, space="DRAM"))
            shard_d = dramp.tile([2, 128, TQ], dt.bfloat16, name="hT_shard_d")
            full_d = dramp.tile([8, 128, TQ], dt.bfloat16, name="hT_full_d")

        # V-aug ones blocks: even head -> cols 64:128, odd head -> cols 0:64
        for j in range(8):
            for h in range(HPC):
                lo = 64 if h % 2 == 0 else 0
                nc.gpsimd.memset(vaug[:, j * HPC + h, lo:lo + 64], 1.0)

        args = (nc, tc, dt, AF, ALU,
                xqT, xkvT, wq, wk, wv, ws1, wo, bq, bk, bv, bs1,
                qscale, qbias, g_all, ebias, dmask, ws2_d, out_d,
                QT, KT, hT, hT_sh, shard_d, full_d, vaug, ctxT)
        if loop_n > 1:
            # Timing variant: body inside a HW loop, pools hoisted outside
            # (PSUM budget trimmed to fit: 2 + 2 + 4 = 8 banks).
            pools = {
                "pp": ctx.enter_context(tc.tile_pool(name="pp", bufs=2, space="PSUM")),
                "scps": ctx.enter_context(tc.tile_pool(name="scps", bufs=2, space="PSUM")),
                "avps": ctx.enter_context(tc.tile_pool(name="avps", bufs=4, space="PSUM")),
                "w2pool": ctx.enter_context(tc.tile_pool(name="w2pool", bufs=3)),
                "epool": ctx.enter_context(tc.tile_pool(name="epool", bufs=3)),
                "npool": ctx.enter_context(tc.tile_pool(name="npool", bufs=2)),
                "opool": ctx.enter_context(tc.tile_pool(name="opool", bufs=3)),
            }
            with tc.For_i(0, loop_n, 1):
                _emit_body(*args, rep=0, pools=pools)
        else:
            for _rep in range(reps):
                _emit_body(*args, rep=_rep)

    _split_excess_waits(nc)
    return nc


def _emit_body(nc, tc, dt, AF, ALU,
               xqT, xkvT, wq, wk, wv, ws1, wo, bq, bk, bv, bs1,
               qscale, qbias, g_all, ebias, dmask, ws2_d, out_d,
               QT, KT, hT, hT_sh, shard_d, full_d, vaug, ctxT,
               rep=0, pools=None):
    from contextlib import ExitStack, nullcontext
    import concourse.mybir as mybir

    # ---- projections ----------------------------------------------------
    pp_cm = (nullcontext(pools["pp"]) if pools is not None
             else tc.tile_pool(name=f"pp{rep}", bufs=6, space="PSUM"))
    with pp_cm as pp:
        # hT first: gelu(ws1^T xqT + bs1).  When SHARD_HT, each core computes
        # a quarter of the sh-tiles and an intra-group AllGather (overlapped
        # with Q/K/V) reassembles the full hT.
        n_st = SH_LOC // 128
        ht_out = hT_sh if SHARD_HT else hT
        for st in range(n_st):
            for qc in range(2):
                ps = pp.tile([128, 512], dt.float32, tag="proj", name="ps_h")
                for Dt in range(8):
                    nc.tensor.matmul(ps[:],
                                     lhsT=ws1[:, Dt, st * 128:(st + 1) * 128],
                                     rhs=xqT[:, Dt, qc * 512:(qc + 1) * 512],
                                     start=(Dt == 0), stop=(Dt == 7))
                nc.scalar.activation(ht_out[:, st, qc * 512:(qc + 1) * 512], ps[:],
                                     AF.Gelu, bias=bs1[:, st:st + 1])
        if SHARD_HT:
            nc.sync.dma_start(out=shard_d.rearrange("s p q -> p s q"), in_=hT_sh[:])
            nc.gpsimd.collective_compute(
                "AllGather", mybir.AluOpType.bypass,
                replica_groups=[[0, 1, 2, 3], [4, 5, 6, 7]],
                ins=[shard_d[:]], outs=[full_d[:]])
            nc.sync.dma_start(out=hT[:], in_=full_d.rearrange("s p q -> p s q"))

        # QT / KT  (weights stationary, x^T moving)
        for name, wmat, xin, outt in (("q", wq, xqT, QT), ("k", wk, xkvT, KT)):
            for t in range(2):
                for qc in range(2):
                    ps = pp.tile([128, 512], dt.float32, tag="proj", name="ps_qk")
                    for Dt in range(8):
                        nc.tensor.matmul(
                            ps[:],
                            lhsT=wmat[:, Dt, t * 128:(t + 1) * 128],
                            rhs=xin[:, Dt, qc * 512:(qc + 1) * 512],
                            start=(Dt == 0), stop=(Dt == 7))
                    if name == "q":
                        nc.scalar.activation(outt[:, t, qc * 512:(qc + 1) * 512],
                                             ps[:], AF.Identity,
                                             bias=qbias[:, t:t + 1],
                                             scale=qscale[:, t:t + 1])
                    else:
                        nc.scalar.activation(outt[:, t, qc * 512:(qc + 1) * 512],
                                             ps[:], AF.Identity,
                                             bias=bk[:, t:t + 1])

        # V natural [k, d] (x^T stationary, weights moving); no bias here.
        for j in range(8):
            ps = pp.tile([128, 512], dt.float32, tag="proj", name="ps_v")
            for Dt in range(8):
                nc.tensor.matmul(ps[:, 0:DCORE],
                                 lhsT=xkvT[:, Dt, j * 128:(j + 1) * 128],
                                 rhs=wv[:, Dt, :], start=(Dt == 0), stop=(Dt == 7))
            pv = ps[:, 0:DCORE].rearrange("p (h d) -> p h d", h=HPC)
            for h in range(HPC):
                lo = 0 if h % 2 == 0 else 64
                nc.vector.tensor_copy(vaug[:, j * HPC + h, lo:lo + 64], pv[:, h, :])

    # ---- attention ------------------------------------------------------
    with ExitStack() as actx:
        if pools is not None:
            scps, avps = pools["scps"], pools["avps"]
            w2pool, epool = pools["w2pool"], pools["epool"]
            npool, opool = pools["npool"], pools["opool"]
        else:
            scps = actx.enter_context(
                tc.tile_pool(name=f"scps{rep}", bufs=3, space="PSUM"))
            avps = actx.enter_context(
                tc.tile_pool(name=f"avps{rep}", bufs=4, space="PSUM"))
            w2pool = actx.enter_context(tc.tile_pool(name=f"w2pool{rep}", bufs=8))
            epool = actx.enter_context(tc.tile_pool(name=f"epool{rep}", bufs=6))
            npool = actx.enter_context(tc.tile_pool(name=f"npool{rep}", bufs=3))
            opool = actx.enter_context(tc.tile_pool(name=f"opool{rep}", bufs=3))

        for h in range(HPC):
            t = h // 2
            r = (h % 2) * 64          # partition offset of this head in QT/KT
            co = (h % 2) * 64         # ctx partition offset (from vaug layout)
            vo = 64 - co              # sums partition offset
            av = [avps.tile([128, 512], dt.float32, tag="av", name=f"av{h}_{i}")
                  for i in range(2)]

            def normalize(qc):
                sc = npool.tile([128, 512], dt.float32, tag="scopy", name="sc_t")
                nc.scalar.copy(sc[vo:vo + 64, :], av[qc][vo:vo + 64, :])
                nc.sync.dma_start(out=sc[co:co + 64, :], in_=sc[vo:vo + 64, :])
                rcp = npool.tile([128, 512], dt.float32, tag="rcp", name="rcp_t")
                nc.vector.reciprocal(rcp[co:co + 64, :], sc[co:co + 64, :])
                cn = npool.tile([128, 512], dt.float32, tag="cn", name="cn_t")
                nc.vector.tensor_mul(cn[co:co + 64, :], av[qc][co:co + 64, :],
                                     rcp[co:co + 64, :])
                nc.scalar.activation(ctxT[co:co + 64, t, qc * 512:(qc + 1) * 512],
                                     cn[co:co + 64, :], AF.Identity,
                                     bias=bv[co:co + 64, t:t + 1])

            for j in range(8):
                w2t = w2pool.tile([128, 8, 128], dt.bfloat16, tag="w2", name="w2t")
                c0 = h * TK + j * 128
                nc.sync.dma_start(
                    out=w2t[:],
                    in_=ws2_d[:, c0:c0 + 128].rearrange("(st p) k -> p st k", p=128))
                for qc in range(2):
                    q0 = max(qc * 512, j * 128)
                    if q0 >= (qc + 1) * 512:
                        continue
                    off = q0 - qc * 512
                    N = 512 - off
                    ps = scps.tile([128, 512], dt.float32, tag="sc", name="ps_sc")
                    nc.tensor.matmul(ps[:, 0:N],
                                     lhsT=KT[r:r + 64, t, j * 128:(j + 1) * 128],
                                     rhs=QT[r:r + 64, t, q0:q0 + N],
                                     start=True, stop=False)
                    for st in range(8):
                        nc.tensor.matmul(ps[:, 0:N], lhsT=w2t[:, st, :],
                                         rhs=hT[:, st, q0:q0 + N],
                                         start=False, stop=(st == 7))
                    if j * 128 >= qc * 512:  # diagonal block present
                        nc.vector.tensor_add(ps[:, 0:128], ps[:, 0:128], dmask[:])
                    et = epool.tile([128, 512], dt.bfloat16, tag="et", name="et")
                    nc.scalar.activation(et[:, 0:N], ps[:, 0:N], AF.Exp,
                                         scale=g_all[:, h:h + 1],
                                         bias=ebias[:, h, j:j + 1])
                    nc.tensor.matmul(av[qc][:, off:512],
                                     lhsT=vaug[:, j * HPC + h, :],
                                     rhs=et[:, 0:N],
                                     start=(j == 0), stop=(j == 3 + 4 * qc))
                if j == 3:
                    normalize(0)   # q-half 0 is complete after k-tile 3
            normalize(1)

        # ---- output projection ------------------------------------------
        for qt in range(8):
            osb = opool.tile([128, DM], dt.float32, tag="osb", name="osb")
            for nc2 in range(2):
                ps = scps.tile([128, 512], dt.float32, tag="sc", name="ps_o")
                for tt in range(2):
                    nc.tensor.matmul(ps[:],
                                     lhsT=ctxT[:, tt, qt * 128:(qt + 1) * 128],
                                     rhs=wo[:, tt, nc2 * 512:(nc2 + 1) * 512],
                                     start=(tt == 0), stop=(tt == 1))
                nc.vector.tensor_copy(osb[:, nc2 * 512:(nc2 + 1) * 512], ps[:])
            nc.sync.dma_start(out=out_d[qt * 128:(qt + 1) * 128, :], in_=osb[:])


def _get_nc(reps=1, loop_n=1):
    key = f"nc{reps}_{loop_n}"
    if key not in _STATE:
        _STATE[key] = _build(reps, loop_n)
    return _STATE[key]


def _make_runner(nc):
    """Build a cached jitted SPMD executor for `nc` (adapted from
    concourse.bass2jax.run_bass_via_pjrt, but reusable across calls)."""
    import jax
    from jax.experimental.shard_map import shard_map
    from jax.sharding import Mesh, PartitionSpec
    from concourse import bass2jax
    import concourse.mybir as mybir

    bass2jax.install_neuronx_cc_hook()
    partition_name = nc.partition_id_tensor.name if nc.partition_id_tensor else None
    in_names, out_names, out_avals, zero_outs = [], [], [], []
    for alloc in nc.m.functions[0].allocations:
        if not isinstance(alloc, mybir.MemoryLocationSet):
            continue
        name = alloc.memorylocations[0].name
        if alloc.kind == "ExternalInput":
            if name != partition_name:
                in_names.append(name)
        elif alloc.kind == "ExternalOutput":
            out_names.append(name)
            shape = tuple(alloc.tensor_shape)
            dtype = mybir.dt.np(alloc.dtype)
            out_avals.append(jax.core.ShapedArray(shape, dtype))
            zero_outs.append(np.zeros(shape, dtype))
    n_params = len(in_names)
    n_outs = len(out_avals)
    all_in = list(in_names) + list(out_names)
    if partition_name is not None:
        all_in.append(partition_name)

    def _body(*args):
        operands = list(args)
        if partition_name is not None:
            operands.append(bass2jax.partition_id_tensor())
        outs = bass2jax._bass_exec_p.bind(
            *operands,
            out_avals=tuple(out_avals),
            in_names=tuple(all_in),
            out_names=tuple(out_names),
            lowering_input_output_aliases=(),
            sim_require_finite=True,
            sim_require_nnan=True,
            nc=nc,
        )
        return tuple(outs)

    devices = jax.devices()[:N_CORES]
    mesh = Mesh(np.asarray(devices), ("core",))
    in_specs = (PartitionSpec("core"),) * (n_params + n_outs)
    out_specs = (PartitionSpec("core"),) * n_outs
    sharded = jax.jit(
        shard_map(_body, mesh=mesh, in_specs=in_specs, out_specs=out_specs,
                  check_rep=False),
        donate_argnums=tuple(range(n_params, n_params + n_outs)),
        keep_unused=True,
    )

    def run(in_maps):
        per_core = [[np.asarray(m[name]) for name in in_names] for m in in_maps]
        concat_in = [
            np.concatenate([per_core[c][i] for c in range(N_CORES)], axis=0)
            for i in range(n_params)
        ]
        concat_zeros = [
            np.zeros((N_CORES * z.shape[0], *z.shape[1:]), z.dtype)
            for z in zero_outs
        ]
        out_arrs = sharded(*concat_in, *concat_zeros)
        return [
            {name: np.asarray(out_arrs[i]).reshape(N_CORES, *out_avals[i].shape)[c]
             for i, name in enumerate(out_names)}
            for c in range(N_CORES)
        ]

    return run


def _get_runner(reps=1, loop_n=1):
    key = f"run{reps}_{loop_n}"
    if key not in _STATE:
        _STATE[key] = _make_runner(_get_nc(reps, loop_n))
    return _STATE[key]


def _pack_small(inputs, kpm_b, cs, ks, ss):
    """Pack bq/bk/bv/bs1/bs2/key_padding_mask into one [128, 54] f32 block.
    Column layout must match the device-side slicing in _build."""
    out = np.zeros((128, 54), np.float32)
    out[:, 0:2] = np.asarray(inputs["bq"], np.float32)[cs].reshape(2, 128).T
    out[:, 2:4] = np.asarray(inputs["bk"], np.float32)[cs].reshape(2, 128).T
    out[:, 4:6] = np.asarray(inputs["bv"], np.float32)[cs].reshape(2, 128).T
    out[:, 6:14] = np.asarray(inputs["bs1"], np.float32)[ss].reshape(-1, 128).T
    bs2 = np.asarray(inputs["bs2"], np.float32)[ks].reshape(HPC, 8, 128)
    out[:, 14:46] = bs2.reshape(32, 128).T
    out[:, 46:54] = kpm_b.astype(np.float32).reshape(8, 128).T
    return out


def _prep_in_maps(inputs):
    x_q = np.asarray(inputs["x_q"], dtype=np.float32)
    x_kv = np.asarray(inputs["x_kv"], dtype=np.float32)
    kpm = np.asarray(inputs["key_padding_mask"], dtype=np.int32)
    Wq = np.asarray(inputs["Wq"], dtype=np.float32)
    Wk = np.asarray(inputs["Wk"], dtype=np.float32)
    Wv = np.asarray(inputs["Wv"], dtype=np.float32)
    Wo = np.asarray(inputs["Wo"], dtype=np.float32)
    Ws1 = np.asarray(inputs["Ws1"], dtype=np.float32)
    Ws2 = np.asarray(inputs["Ws2"], dtype=np.float32)

    xT = [np.ascontiguousarray(x_q[b].T).astype(BF16) for b in range(B)]
    xkT = [np.ascontiguousarray(x_kv[b].T).astype(BF16) for b in range(B)]

    in_maps = []
    for core in range(N_CORES):
        b, g = core // 4, core % 4
        cs = slice(g * DCORE, (g + 1) * DCORE)
        ks = slice(g * KC, (g + 1) * KC)
        ss = slice(g * SH_LOC, (g + 1) * SH_LOC) if SHARD_HT else slice(0, SH)
        in_maps.append({
            "xqT": xT[b],
            "xkvT": xkT[b],
            "wq": np.ascontiguousarray(Wq[:, cs]).astype(BF16),
            "wk": np.ascontiguousarray(Wk[:, cs]).astype(BF16),
            "wv": np.ascontiguousarray(Wv[:, cs]).astype(BF16),
            "ws1": np.ascontiguousarray(Ws1[:, ss]).astype(BF16),
            "ws2": np.ascontiguousarray(Ws2[:, ks]).astype(BF16),
            "wo": np.ascontiguousarray(Wo[cs, :]).astype(BF16),
            "small": _pack_small(inputs, kpm[b], cs, ks, ss),
            "gates": np.ascontiguousarray(
                np.asarray(inputs["gate"], np.float32)[g * HPC:(g + 1) * HPC]
            ).reshape(1, HPC),
        })
    return in_maps


def kernel(**inputs):
    run = _get_runner(1)
    in_maps = _prep_in_maps(inputs)
    results = run(in_maps)
    bo = np.asarray(inputs["bo"], dtype=np.float32)
    out = np.zeros((B, TQ, DM), dtype=np.float32)
    for b in range(B):
        acc = np.tile(bo[None, :], (TQ, 1))
        for g in range(4):
            acc += results[b * 4 + g]["out"]
        out[b] = acc
    return out


# revision 74
# speedup vs baseline: 3.5799x; 3.5799x over previous
"""Trainium2 Bass kernel for nn_PluggableMHA (hybrid dense-synthesizer MHA).

Sharding: 8 cores = (batch b in {0,1}) x (head-group g in {0..3}, 4 heads each).
Each core computes, for its batch and its 4 heads:
    Q/K projections in transposed layout  QT/KT [d, t]
    V projection in natural layout        V  [k, d]   (+ all-ones block)
    hT = gelu(Ws1^T x_q^T + bs1)          [sh, q]
    scoresT[k, q] = s_h*(K Q^T) + (h^T Ws2)^T    accumulated in PSUM
    expT = Exp(g_h * scoresT + g_h*bs2 + pad)    (causal handled by trimming
                                                  + triangular mask tile)
    ctx'^T, sums = V_aug^T expT   (ones column block gives softmax sums)
    ctxT = ctx'^T / sums + bv
    partial_out = ctxT^T Wo_rows
The host sums the 4 head-group partials per batch and adds bo.

All matmuls run in bf16 (fp32 PSUM accumulation).
"""

import sys

if "/opt/trn_rl_repo" not in sys.path:
    sys.path.insert(0, "/opt/trn_rl_repo")

import numpy as np
import ml_dtypes

BF16 = ml_dtypes.bfloat16

# Problem shapes (hardcoded per contract).
B = 2
TQ = 1024
TK = 1024
DM = 1024          # d_model
NH = 16            # total heads
DH = 64            # head dim
SH = 1024          # synth hidden
HPC = 4            # heads per core
DCORE = HPC * DH   # 256 q/k/v columns per core
KC = HPC * TK      # 4096 ws2 columns per core
N_CORES = 8
NEG = -1e30
ATT_SCALE = DH ** -0.5

# Shard the synthesizer-hidden projection across the 4 cores of a batch and
# AllGather the result.  Saves ~25us of PE work per core, but in-NEFF
# collectives proved unreliable on the axon PJRT path (device unrecoverable),
# so this stays off.
SHARD_HT = False
SH_LOC = SH // 4 if SHARD_HT else SH

_STATE: dict = {}


def _patch_tile_drain():
    """This neuronxcc's Drain lowering supports fewer sync-wait slots than
    Tile's exit drain carries; split the waits across single-wait SP nops."""
    import bass_rust
    import concourse.tile as tile_mod
    from concourse.vector_clock import ScopedClock

    if getattr(tile_mod.TileContext, "_drain_patched", False):
        return

    def _drain_and_barrier(self, tick_clock, wait_clock):
        nc = self.nc
        drain_inst = nc.sync.drain()
        wait_clock.add_sem_waits(
            drain_inst.ins, ScopedClock({None: tick_clock.global_clock})
        )
        si = drain_inst.ins.sync_info
        waits = list(si.on_wait) if si is not None else []
        ups = list(si.on_update) if si is not None else []
        if len(waits) > 1:
            drain_inst.ins.sync_info = bass_rust.SyncInfo(on_wait=[], on_update=ups)
            for w in waits:
                n = nc.sync.nop(nofuse=True)
                n.ins.sync_info = bass_rust.SyncInfo(on_wait=[w], on_update=[])
        nc.all_engine_barrier()
        popped = nc._tile_sem_poison_stack.pop()
        assert popped is self._sem_poison
        nc.clear_and_free_semaphores(list(self.sems.allocated().values()))
        nc.all_engine_barrier()

    tile_mod.TileContext._drain_and_barrier = _drain_and_barrier
    tile_mod.TileContext._drain_patched = True


def _split_excess_waits(nc, max_waits=1):
    """This neuronxcc build supports only a small number of sync-wait slots
    per instruction; hoist excess waits onto prepended same-engine nops."""
    import concourse.mybir as mybir
    import bass_rust

    cnt = 0

    def fix_block(b):
        nonlocal cnt
        new_insts = []
        changed = False
        for inst in b.instructions:
            si = inst.sync_info
            waits = list(si.on_wait) if si is not None else []
            if len(waits) > max_waits:
                extra, keep = waits[:-max_waits], waits[-max_waits:]
                for w in extra:
                    n = mybir.InstNoOp(name=f"{inst.name}-wsplit{cnt}", ins=[], outs=[])
                    cnt += 1
                    n.engine = inst.engine
                    n.sync_info = bass_rust.SyncInfo(on_wait=[w], on_update=[])
                    new_insts.append(n)
                inst.sync_info = bass_rust.SyncInfo(
                    on_wait=keep, on_update=list(si.on_update))
                changed = True
            new_insts.append(inst)
        if changed:
            b.instructions = new_insts
        for sb in getattr(b, "blocks", None) or []:
            fix_block(sb)

    for f in nc.m.functions:
        for b in f.blocks:
            fix_block(b)
    return cnt


def _build(reps=1, loop_n=1):
    import concourse.bass as bass
    import concourse.tile as tile
    import concourse.mybir as mybir
    from contextlib import ExitStack

    _patch_tile_drain()

    dt = mybir.dt
    AF = mybir.ActivationFunctionType
    ALU = mybir.AluOpType

    nc = bass.Bass()

    # ---- DRAM I/O -------------------------------------------------------
    xqT_d = nc.declare_dram_parameter("xqT", [DM, TQ], dt.bfloat16, isOutput=False)
    xkvT_d = nc.declare_dram_parameter("xkvT", [DM, TK], dt.bfloat16, isOutput=False)
    wq_d = nc.declare_dram_parameter("wq", [DM, DCORE], dt.bfloat16, isOutput=False)
    wk_d = nc.declare_dram_parameter("wk", [DM, DCORE], dt.bfloat16, isOutput=False)
    wv_d = nc.declare_dram_parameter("wv", [DM, DCORE], dt.bfloat16, isOutput=False)
    ws1_d = nc.declare_dram_parameter("ws1", [DM, SH_LOC], dt.bfloat16, isOutput=False)
    ws2_d = nc.declare_dram_parameter("ws2", [SH, KC], dt.bfloat16, isOutput=False)
    wo_d = nc.declare_dram_parameter("wo", [DCORE, DM], dt.bfloat16, isOutput=False)
    # small tensors packed host-side into one [128, 54] f32 block:
    # cols 0:2 bq, 2:4 bk, 4:6 bv, 6:14 bs1, 14:46 bs2 (h-major), 46:54 kpm(f32)
    small_d = nc.declare_dram_parameter("small", [128, 54], dt.float32, isOutput=False)
    gates_d = nc.declare_dram_parameter("gates", [1, HPC], dt.float32, isOutput=False)
    out_d = nc.declare_dram_parameter("out", [TQ, DM], dt.float32, isOutput=True)

    with tile.TileContext(nc) as tc, ExitStack() as ctx:
        const = ctx.enter_context(tc.tile_pool(name="const", bufs=1))

        def ctile(shape, dtype, tag):
            return const.tile(shape, dtype, tag=tag, name=tag)

        # ---- resident SBUF tensors --------------------------------------
        # DMA'd in per-Dt slices, emitted in consumption order, so the
        # projection matmuls can start as soon as their first slices land.
        gate_in = ctile([128, HPC], dt.float32, "gate_in")
        nc.sync.dma_start(out=gate_in[0:1, :], in_=gates_d[:])

        xqT = ctile([128, 8, TQ], dt.bfloat16, "xqT")
        xkvT = ctile([128, 8, TK], dt.bfloat16, "xkvT")
        wq = ctile([128, 8, DCORE], dt.bfloat16, "wq")
        wk = ctile([128, 8, DCORE], dt.bfloat16, "wk")
        wv = ctile([128, 8, DCORE], dt.bfloat16, "wv")
        ws1 = ctile([128, 8, SH_LOC], dt.bfloat16, "ws1")
        xqT_r = xqT_d.rearrange("(t p) q -> p t q", p=128)
        xkvT_r = xkvT_d.rearrange("(t p) q -> p t q", p=128)
        wq_r = wq_d.rearrange("(t p) c -> p t c", p=128)
        wk_r = wk_d.rearrange("(t p) c -> p t c", p=128)
        wv_r = wv_d.rearrange("(t p) c -> p t c", p=128)
        ws1_r = ws1_d.rearrange("(t p) s -> p t s", p=128)
        for Dt in range(8):
            nc.sync.dma_start(out=ws1[:, Dt, :], in_=ws1_r[:, Dt, :])
            nc.sync.dma_start(out=xqT[:, Dt, :], in_=xqT_r[:, Dt, :])
            if Dt == 7:
                small = ctile([128, 54], dt.float32, "small")
                nc.sync.dma_start(out=small[:], in_=small_d[:])
        bq = small[:, 0:2]
        bk = small[:, 2:4]
        bv = small[:, 4:6]
        bs1 = small[:, 6:14]
        bs2 = small[:, 14:46].rearrange("p (h j) -> p h j", h=HPC)
        kpmf = small[:, 46:54]
        nc.sync.dma_start(out=wq[:], in_=wq_r[:])
        for half in range(2):
            nc.sync.dma_start(out=wk[:, half * 4:(half + 1) * 4, :],
                              in_=wk_r[:, half * 4:(half + 1) * 4, :])
            nc.sync.dma_start(out=xkvT[:, half * 4:(half + 1) * 4, :],
                              in_=xkvT_r[:, half * 4:(half + 1) * 4, :])
        nc.sync.dma_start(out=wv[:], in_=wv_r[:])
        wo = ctile([128, 2, DM], dt.bfloat16, "wo")
        nc.sync.dma_start(out=wo[:], in_=wo_d.rearrange("(t p) n -> p t n", p=128))

        # ---- setup: per-head scalars, masks -----------------------------
        ones_t = ctile([128, 128], dt.float32, "ones_t")
        nc.vector.memset(ones_t[:], 1.0)

        with tc.tile_pool(name="gpp", bufs=1, space="PSUM") as gpp:
            ps_g = gpp.tile([128, 512], dt.float32, tag="gproj", name="ps_g")
            nc.tensor.matmul(ps_g[:, 0:HPC], lhsT=ones_t[0:1, :],
                             rhs=gate_in[0:1, :], start=True, stop=True)
            g_all = ctile([128, HPC], dt.float32, "g_all")   # sigmoid(gate), bcast
            nc.scalar.activation(g_all[:], ps_g[:, 0:HPC], AF.Sigmoid)

        r_all = ctile([128, HPC], dt.float32, "r_all")       # 1/g
        nc.vector.reciprocal(r_all[:], g_all[:])
        s_all = ctile([128, HPC], dt.float32, "s_all")       # 0.125*(1-g)/g
        nc.vector.tensor_scalar(s_all[:], r_all[:], ATT_SCALE, -ATT_SCALE,
                                op0=ALU.mult, op1=ALU.add)

        qscale = ctile([128, 2], dt.float32, "qscale")       # per-partition Q scale
        for t in range(2):
            nc.vector.tensor_copy(qscale[0:64, t:t + 1], s_all[0:64, 2 * t:2 * t + 1])
            nc.vector.tensor_copy(qscale[64:128, t:t + 1],
                                  s_all[64:128, 2 * t + 1:2 * t + 2])
        qbias = ctile([128, 2], dt.float32, "qbias")         # bq * qscale
        nc.vector.tensor_mul(qbias[:], bq[:], qscale[:])

        pad_add = ctile([128, 8], dt.float32, "pad_add")     # 0 keep / NEG masked
        nc.vector.tensor_scalar(pad_add[:], kpmf[:], -NEG, NEG,
                                op0=ALU.mult, op1=ALU.add)
        ebias = ctile([128, 4, 8], dt.float32, "ebias")      # g*bs2 + pad
        for h in range(HPC):
            nc.vector.scalar_tensor_tensor(
                out=ebias[:, h, :], in0=bs2[:, h, :], scalar=g_all[:, h:h + 1],
                in1=pad_add[:], op0=ALU.mult, op1=ALU.add)

        dmask = ctile([128, 128], dt.float32, "dmask")       # scoresT tri mask
        nc.gpsimd.memset(dmask[:], 0.0)
        # keep (0) where q - k >= 0, else NEG   (p = k, f = q within tile)
        nc.gpsimd.affine_select(out=dmask[:], in_=dmask[:],
                                compare_op=ALU.is_ge, fill=NEG, base=0,
                                pattern=[[1, 128]], channel_multiplier=-1)

        # ---- persistent compute tensors ---------------------------------
        QT = ctile([128, 2, TQ], dt.bfloat16, "QT")
        KT = ctile([128, 2, TK], dt.bfloat16, "KT")
        hT = ctile([128, 8, TQ], dt.bfloat16, "hT")
        hT_sh = ctile([128, 2, TQ], dt.bfloat16, "hT_sh")
        vaug = ctile([128, 32, 128], dt.bfloat16, "vaug")
        ctxT = ctile([128, 2, TQ], dt.bfloat16, "ctxT")

        shard_d = full_d = None
        if SHARD_HT:
            dramp = ctx.enter_context(tc.tile_pool(name="dramp", bufs=1, space="DRAM"))
            shard_d = dramp.tile([2, 128, TQ], dt.bfloat16, name="hT_shard_d")
            full_d = dramp.tile([8, 128, TQ], dt.bfloat16, name="hT_full_d")

        # V-aug ones blocks: even head -> cols 64:128, odd head -> cols 0:64
        for j in range(8):
            for h in range(HPC):
                lo = 64 if h % 2 == 0 else 0
                nc.gpsimd.memset(vaug[:, j * HPC + h, lo:lo + 64], 1.0)

        args = (nc, tc, dt, AF, ALU,
                xqT, xkvT, wq, wk, wv, ws1, wo, bq, bk, bv, bs1,
                qscale, qbias, g_all, ebias, dmask, ws2_d, out_d,
                QT, KT, hT, hT_sh, shard_d, full_d, vaug, ctxT)
        if loop_n > 1:
            # Timing variant: body inside a HW loop, pools hoisted outside
            # (PSUM budget trimmed to fit: 2 + 2 + 4 = 8 banks).
            pools = {
                "pp": ctx.enter_context(tc.tile_pool(name="pp", bufs=2, space="PSUM")),
                "scps": ctx.enter_context(tc.tile_pool(name="scps", bufs=2, space="PSUM")),
                "avps": ctx.enter_context(tc.tile_pool(name="avps", bufs=4, space="PSUM")),
                "w2pool": ctx.enter_context(tc.tile_pool(name="w2pool", bufs=3)),
                "epool": ctx.enter_context(tc.tile_pool(name="epool", bufs=3)),
                "npool": ctx.enter_context(tc.tile_pool(name="npool", bufs=2)),
                "opool": ctx.enter_context(tc.tile_pool(name="opool", bufs=3)),
            }
            with tc.For_i(0, loop_n, 1):
                _emit_body(*args, rep=0, pools=pools)
        else:
            for _rep in range(reps):
                _emit_body(*args, rep=_rep)

    _split_excess_waits(nc)
    return nc


def _emit_body(nc, tc, dt, AF, ALU,
               xqT, xkvT, wq, wk, wv, ws1, wo, bq, bk, bv, bs1,
               qscale, qbias, g_all, ebias, dmask, ws2_d, out_d,
               QT, KT, hT, hT_sh, shard_d, full_d, vaug, ctxT,
               rep=0, pools=None):
    from contextlib import ExitStack, nullcontext
    import concourse.mybir as mybir

    # ---- projections ----------------------------------------------------
    pp_cm = (nullcontext(pools["pp"]) if pools is not None
             else tc.tile_pool(name=f"pp{rep}", bufs=6, space="PSUM"))
    with pp_cm as pp:
        # hT first: gelu(ws1^T xqT + bs1).  When SHARD_HT, each core computes
        # a quarter of the sh-tiles and an intra-group AllGather (overlapped
        # with Q/K/V) reassembles the full hT.
        n_st = SH_LOC // 128
        ht_out = hT_sh if SHARD_HT else hT
        for st in range(n_st):
            for qc in range(2):
                ps = pp.tile([128, 512], dt.float32, tag="proj", name="ps_h")
                for Dt in range(8):
                    nc.tensor.matmul(ps[:],
                                     lhsT=ws1[:, Dt, st * 128:(st + 1) * 128],
                                     rhs=xqT[:, Dt, qc * 512:(qc + 1) * 512],
                                     start=(Dt == 0), stop=(Dt == 7))
                nc.scalar.activation(ht_out[:, st, qc * 512:(qc + 1) * 512], ps[:],
                                     AF.Gelu, bias=bs1[:, st:st + 1])
        if SHARD_HT:
            nc.sync.dma_start(out=shard_d.rearrange("s p q -> p s q"), in_=hT_sh[:])
            nc.gpsimd.collective_compute(
                "AllGather", mybir.AluOpType.bypass,
                replica_groups=[[0, 1, 2, 3], [4, 5, 6, 7]],
                ins=[shard_d[:]], outs=[full_d[:]])
            nc.sync.dma_start(out=hT[:], in_=full_d.rearrange("s p q -> p s q"))

        # QT / KT  (weights stationary, x^T moving)
        for name, wmat, xin, outt in (("q", wq, xqT, QT), ("k", wk, xkvT, KT)):
            for t in range(2):
                for qc in range(2):
                    ps = pp.tile([128, 512], dt.float32, tag="proj", name="ps_qk")
                    for Dt in range(8):
                        nc.tensor.matmul(
                            ps[:],
                            lhsT=wmat[:, Dt, t * 128:(t + 1) * 128],
                            rhs=xin[:, Dt, qc * 512:(qc + 1) * 512],
                            start=(Dt == 0), stop=(Dt == 7))
                    if name == "q":
                        nc.scalar.activation(outt[:, t, qc * 512:(qc + 1) * 512],
                                             ps[:], AF.Identity,
                                             bias=qbias[:, t:t + 1],
                                             scale=qscale[:, t:t + 1])
                    else:
                        nc.scalar.activation(outt[:, t, qc * 512:(qc + 1) * 512],
                                             ps[:], AF.Identity,
                                             bias=bk[:, t:t + 1])

        # V natural [k, d] (x^T stationary, weights moving); no bias here.
        for j in range(8):
            ps = pp.tile([128, 512], dt.float32, tag="proj", name="ps_v")
            for Dt in range(8):
                nc.tensor.matmul(ps[:, 0:DCORE],
                                 lhsT=xkvT[:, Dt, j * 128:(j + 1) * 128],
                                 rhs=wv[:, Dt, :], start=(Dt == 0), stop=(Dt == 7))
            pv = ps[:, 0:DCORE].rearrange("p (h d) -> p h d", h=HPC)
            for h in range(HPC):
                lo = 0 if h % 2 == 0 else 64
                nc.vector.tensor_copy(vaug[:, j * HPC + h, lo:lo + 64], pv[:, h, :])

    # ---- attention ------------------------------------------------------
    with ExitStack() as actx:
        if pools is not None:
            scps, avps = pools["scps"], pools["avps"]
            w2pool, epool = pools["w2pool"], pools["epool"]
            npool, opool = pools["npool"], pools["opool"]
        else:
            scps = actx.enter_context(
                tc.tile_pool(name=f"scps{rep}", bufs=3, space="PSUM"))
            avps = actx.enter_context(
                tc.tile_pool(name=f"avps{rep}", bufs=4, space="PSUM"))
            w2pool = actx.enter_context(tc.tile_pool(name=f"w2pool{rep}", bufs=8))
            epool = actx.enter_context(tc.tile_pool(name=f"epool{rep}", bufs=6))
            npool = actx.enter_context(tc.tile_pool(name=f"npool{rep}", bufs=3))
            opool = actx.enter_context(tc.tile_pool(name=f"opool{rep}", bufs=3))

        for h in range(HPC):
            t = h // 2
            r = (h % 2) * 64          # partition offset of this head in QT/KT
            co = (h % 2) * 64         # ctx partition offset (from vaug layout)
            vo = 64 - co              # sums partition offset
            av = [avps.tile([128, 512], dt.float32, tag="av", name=f"av{h}_{i}")
                  for i in range(2)]

            def normalize(qc):
                sc = npool.tile([128, 512], dt.float32, tag="scopy", name="sc_t")
                nc.scalar.copy(sc[vo:vo + 64, :], av[qc][vo:vo + 64, :])
                nc.sync.dma_start(out=sc[co:co + 64, :], in_=sc[vo:vo + 64, :])
                rcp = npool.tile([128, 512], dt.float32, tag="rcp", name="rcp_t")
                nc.vector.reciprocal(rcp[co:co + 64, :], sc[co:co + 64, :])
                cn = npool.tile([128, 512], dt.float32, tag="cn", name="cn_t")
                nc.vector.tensor_mul(cn[co:co + 64, :], av[qc][co:co + 64, :],
                                     rcp[co:co + 64, :])
                nc.scalar.activation(ctxT[co:co + 64, t, qc * 512:(qc + 1) * 512],
                                     cn[co:co + 64, :], AF.Identity,
                                     bias=bv[co:co + 64, t:t + 1])

            for j in range(8):
                w2t = w2pool.tile([128, 8, 128], dt.bfloat16, tag="w2", name="w2t")
                c0 = h * TK + j * 128
                nc.sync.dma_start(
                    out=w2t[:],
                    in_=ws2_d[:, c0:c0 + 128].rearrange("(st p) k -> p st k", p=128))
                for qc in range(2):
                    q0 = max(qc * 512, j * 128)
                    if q0 >= (qc + 1) * 512:
                        continue
                    off = q0 - qc * 512
                    N = 512 - off
                    ps = scps.tile([128, 512], dt.float32, tag="sc", name="ps_sc")
                    nc.tensor.matmul(ps[:, 0:N],
                                     lhsT=KT[r:r + 64, t, j * 128:(j + 1) * 128],
                                     rhs=QT[r:r + 64, t, q0:q0 + N],
                                     start=True, stop=False)
                    for st in range(8):
                        nc.tensor.matmul(ps[:, 0:N], lhsT=w2t[:, st, :],
                                         rhs=hT[:, st, q0:q0 + N],
                                         start=False, stop=(st == 7))
                    if j * 128 >= qc * 512:  # diagonal block present
                        nc.vector.tensor_add(ps[:, 0:128], ps[:, 0:128], dmask[:])
                    et = epool.tile([128, 512], dt.bfloat16, tag="et", name="et")
                    nc.scalar.activation(et[:, 0:N], ps[:, 0:N], AF.Exp,
                                         scale=g_all[:, h:h + 1],
                                         bias=ebias[:, h, j:j + 1])
                    nc.tensor.matmul(av[qc][:, off:512],
                                     lhsT=vaug[:, j * HPC + h, :],
                                     rhs=et[:, 0:N],
                                     start=(j == 0), stop=(j == 3 + 4 * qc))
                if j == 3:
                    normalize(0)   # q-half 0 is complete after k-tile 3
            normalize(1)

        # ---- output projection ------------------------------------------
        for qt in range(8):
            osb = opool.tile([128, DM], dt.float32, tag="osb", name="osb")
            for nc2 in range(2):
                ps = scps.tile([128, 512], dt.float32, tag="sc", name="ps_o")
                for tt in range(2):
                    nc.tensor.matmul(ps[:],
                                     lhsT=ctxT[:, tt, qt * 128:(qt + 1) * 128],
                                     rhs=wo[:, tt, nc2 * 512:(nc2 + 1) * 512],
                                     start=(tt == 0), stop=(tt == 1))
                nc.vector.tensor_copy(osb[:, nc2 * 512:(nc2 + 1) * 512], ps[:])
                # per-chunk store so the last q-tiles' output drains overlap
                # the remaining Wo matmuls
                nc.sync.dma_start(
                    out=out_d[qt * 128:(qt + 1) * 128, nc2 * 512:(nc2 + 1) * 512],
                    in_=osb[:, nc2 * 512:(nc2 + 1) * 512])


def _get_nc(reps=1, loop_n=1):
    key = f"nc{reps}_{loop_n}"
    if key not in _STATE:
        _STATE[key] = _build(reps, loop_n)
    return _STATE[key]


def _make_runner(nc):
    """Build a cached jitted SPMD executor for `nc` (adapted from
    concourse.bass2jax.run_bass_via_pjrt, but reusable across calls)."""
    import jax
    from jax.experimental.shard_map import shard_map
    from jax.sharding import Mesh, PartitionSpec
    from concourse import bass2jax
    import concourse.mybir as mybir

    bass2jax.install_neuronx_cc_hook()
    partition_name = nc.partition_id_tensor.name if nc.partition_id_tensor else None
    in_names, out_names, out_avals, zero_outs = [], [], [], []
    for alloc in nc.m.functions[0].allocations:
        if not isinstance(alloc, mybir.MemoryLocationSet):
            continue
        name = alloc.memorylocations[0].name
        if alloc.kind == "ExternalInput":
            if name != partition_name:
                in_names.append(name)
        elif alloc.kind == "ExternalOutput":
            out_names.append(name)
            shape = tuple(alloc.tensor_shape)
            dtype = mybir.dt.np(alloc.dtype)
            out_avals.append(jax.core.ShapedArray(shape, dtype))
            zero_outs.append(np.zeros(shape, dtype))
    n_params = len(in_names)
    n_outs = len(out_avals)
    all_in = list(in_names) + list(out_names)
    if partition_name is not None:
        all_in.append(partition_name)

    def _body(*args):
        operands = list(args)
        if partition_name is not None:
            operands.append(bass2jax.partition_id_tensor())
        outs = bass2jax._bass_exec_p.bind(
            *operands,
            out_avals=tuple(out_avals),
            in_names=tuple(all_in),
            out_names=tuple(out_names),
            lowering_input_output_aliases=(),
            sim_require_finite=True,
            sim_require_nnan=True,
            nc=nc,
        )
        return tuple(outs)

    devices = jax.devices()[:N_CORES]
    mesh = Mesh(np.asarray(devices), ("core",))
    in_specs = (PartitionSpec("core"),) * (n_params + n_outs)
    out_specs = (PartitionSpec("core"),) * n_outs
    sharded = jax.jit(
        shard_map(_body, mesh=mesh, in_specs=in_specs, out_specs=out_specs,
                  check_rep=False),
        donate_argnums=tuple(range(n_params, n_params + n_outs)),
        keep_unused=True,
    )

    def run(in_maps):
        per_core = [[np.asarray(m[name]) for name in in_names] for m in in_maps]
        concat_in = [
            np.concatenate([per_core[c][i] for c in range(N_CORES)], axis=0)
            for i in range(n_params)
        ]
        concat_zeros = [
            np.zeros((N_CORES * z.shape[0], *z.shape[1:]), z.dtype)
            for z in zero_outs
        ]
        out_arrs = sharded(*concat_in, *concat_zeros)
        return [
            {name: np.asarray(out_arrs[i]).reshape(N_CORES, *out_avals[i].shape)[c]
             for i, name in enumerate(out_names)}
            for c in range(N_CORES)
        ]

    return run


def _get_runner(reps=1, loop_n=1):
    key = f"run{reps}_{loop_n}"
    if key not in _STATE:
        _STATE[key] = _make_runner(_get_nc(reps, loop_n))
    return _STATE[key]


def _pack_small(inputs, kpm_b, cs, ks, ss):
    """Pack bq/bk/bv/bs1/bs2/key_padding_mask into one [128, 54] f32 block.
    Column layout must match the device-side slicing in _build."""
    out = np.zeros((128, 54), np.float32)
    out[:, 0:2] = np.asarray(inputs["bq"], np.float32)[cs].reshape(2, 128).T
    out[:, 2:4] = np.asarray(inputs["bk"], np.float32)[cs].reshape(2, 128).T
    out[:, 4:6] = np.asarray(inputs["bv"], np.float32)[cs].reshape(2, 128).T
    out[:, 6:14] = np.asarray(inputs["bs1"], np.float32)[ss].reshape(-1, 128).T
    bs2 = np.asarray(inputs["bs2"], np.float32)[ks].reshape(HPC, 8, 128)
    out[:, 14:46] = bs2.reshape(32, 128).T
    out[:, 46:54] = kpm_b.astype(np.float32).reshape(8, 128).T
    return out


def _prep_in_maps(inputs):
    x_q = np.asarray(inputs["x_q"], dtype=np.float32)
    x_kv = np.asarray(inputs["x_kv"], dtype=np.float32)
    kpm = np.asarray(inputs["key_padding_mask"], dtype=np.int32)
    Wq = np.asarray(inputs["Wq"], dtype=np.float32)
    Wk = np.asarray(inputs["Wk"], dtype=np.float32)
    Wv = np.asarray(inputs["Wv"], dtype=np.float32)
    Wo = np.asarray(inputs["Wo"], dtype=np.float32)
    Ws1 = np.asarray(inputs["Ws1"], dtype=np.float32)
    Ws2 = np.asarray(inputs["Ws2"], dtype=np.float32)

    xT = [np.ascontiguousarray(x_q[b].T).astype(BF16) for b in range(B)]
    xkT = [np.ascontiguousarray(x_kv[b].T).astype(BF16) for b in range(B)]

    in_maps = []
    for core in range(N_CORES):
        b, g = core // 4, core % 4
        cs = slice(g * DCORE, (g + 1) * DCORE)
        ks = slice(g * KC, (g + 1) * KC)
        ss = slice(g * SH_LOC, (g + 1) * SH_LOC) if SHARD_HT else slice(0, SH)
        in_maps.append({
            "xqT": xT[b],
            "xkvT": xkT[b],
            "wq": np.ascontiguousarray(Wq[:, cs]).astype(BF16),
            "wk": np.ascontiguousarray(Wk[:, cs]).astype(BF16),
            "wv": np.ascontiguousarray(Wv[:, cs]).astype(BF16),
            "ws1": np.ascontiguousarray(Ws1[:, ss]).astype(BF16),
            "ws2": np.ascontiguousarray(Ws2[:, ks]).astype(BF16),
            "wo": np.ascontiguousarray(Wo[cs, :]).astype(BF16),
            "small": _pack_small(inputs, kpm[b], cs, ks, ss),
            "gates": np.ascontiguousarray(
                np.asarray(inputs["gate"], np.float32)[g * HPC:(g + 1) * HPC]
            ).reshape(1, HPC),
        })
    return in_maps


def kernel(**inputs):
    run = _get_runner(1)
    in_maps = _prep_in_maps(inputs)
    results = run(in_maps)
    bo = np.asarray(inputs["bo"], dtype=np.float32)
    out = np.zeros((B, TQ, DM), dtype=np.float32)
    for b in range(B):
        acc = np.tile(bo[None, :], (TQ, 1))
        for g in range(4):
            acc += results[b * 4 + g]["out"]
        out[b] = acc
    return out
